# revision 26
# baseline (speedup 1.0000x reference)
"""Trainium2 Bass kernel for nn_AntecedentScore (coref antecedent scoring).

Reference computation (K=1200 spans, C=150 antecedents, D=500, F=20, H=150):
    pair_emb[i,c] = [tgt_i, ante_j, tgt_i*ante_j, dist_emb[bucket(i-j)]]   (j=antecedents[i,c])
    scores = FFNN3(pair_emb) + ms[i] + ms[j]
    returns (scores, antecedent_emb, pair_emb)

Strategy: shard the k (top-span) dimension across 8 cores (150 rows each,
22500 pairs -> 176 tiles of 128 pairs).  Host precomputes the linear FFNN
terms that don't depend on the per-pair product:
    pair@W_in = tgt@W1 + ante@W2 + sim@W3 + feat@W4
and everything except sim@W3 collapses into a per-pair gathered sum PAIRSUM.
Device, per 256-pair double-tile:
  - target rows arrive via broadcast-read DMAs from a per-core local span
    table (each span's row replicated across its antecedent rows),
  - antecedent rows via indirect-DMA gather from the replicated span table,
  - DVE multiplies for sim; one contiguous 1.5 MB DMA writes the pair tile
    (the dominant, unavoidable HBM traffic),
  - per 512-pair supertile the PE transposes sim and runs
    sim@W3 (+PAIRSUM, relu) -> W_h (+b, relu) -> W_out (+ms) at N=512,
    producing scores.
"""

import numpy as np

# ---------------------------------------------------------------- constants
K, C, D, F, H = 1200, 150, 500, 20, 150
NCORES = 8
KLOC = K // NCORES            # 150 spans per core
NPAIR = KLOC * C              # 22500 pairs per core
P = 128
NT = (NPAIR + P - 1) // P     # 176 pair tiles per core
NPAD = NT * P                 # 22528
W = 3 * D + F                 # 1520 pair_emb width
WB = 3 * D                    # 1500 = device-written pair block width
HP = 2 * P                    # 256 = padded hidden size
ST = 4                        # tiles per supertile (FFNN batch: N=512 pairs)
NS = NT // ST                 # 44 supertiles
SN = ST * P                   # 512
# sim contraction chunks of width 128 covering D=500 (last chunk overlaps;
# the overlapped weight rows are zeroed in chunk 3 so nothing double-counts)
CH = [0, 128, 256, D - P]     # [0, 128, 256, 372]
OVL = 256 + P - CH[3]         # = 12 rows double-covered

_NC_CACHE = {}


# ------------------------------------------------------------ host packing
def _bucket_distance(d):
    with np.errstate(divide="ignore"):
        log_idx = (np.floor(np.log2(np.maximum(d, 1).astype(np.float32))) + 3).astype(
            d.dtype
        )
    return np.clip(np.where(d <= 4, d, log_idx), 0, 9)


def host_prep(inputs):
    """Returns per_core_inputs[8] numpy dicts."""
    span = np.ascontiguousarray(np.asarray(inputs["top_span_emb"], np.float32))
    ms = np.asarray(inputs["top_span_mention_scores"], np.float32)
    ante = np.asarray(inputs["antecedents"]).astype(np.int64)
    dist_emb = np.asarray(inputs["dist_emb"], np.float32)
    W_in = np.asarray(inputs["W_in"], np.float32)
    b_in = np.asarray(inputs["b_in"], np.float32)
    W_h = np.asarray(inputs["W_h"], np.float32)
    b_h = np.asarray(inputs["b_h"], np.float32)
    W_out = np.asarray(inputs["W_out"], np.float32)
    b_out = np.asarray(inputs["b_out"], np.float32)

    W1, W2, W3, W4 = W_in[:D], W_in[D : 2 * D], W_in[2 * D : 3 * D], W_in[3 * D :]

    TA = span @ W1                      # [K, H]
    AA = span @ W2                      # [K, H]
    FA = dist_emb @ W4                  # [10, H]

    dist = np.arange(K, dtype=ante.dtype)[:, None] - ante
    bucket = _bucket_distance(dist)     # [K, C]

    # w3 chunks [4, 128, HP]; chunk 3 overlaps chunk 2 by OVL rows -> zero them
    w3c = np.zeros((4, P, HP), np.float32)
    for c in range(4):
        w3c[c, :, :H] = W3[CH[c] : CH[c] + P]
    w3c[3, :OVL, :] = 0.0

    whc = np.zeros((2, P, HP), np.float32)
    whc[0, :, :H] = W_h[:P]
    whc[1, : H - P, :H] = W_h[P:]

    woc = np.zeros((2, P, 1), np.float32)
    woc[0, :, 0] = W_out[:P, 0]
    woc[1, : H - P, 0] = W_out[P:, 0]

    bh = np.zeros((P, 2), np.float32)
    bh[:, 0] = b_h[:P]
    bh[: H - P, 1] = b_h[P:]

    ident = np.eye(P, dtype=np.float32)

    per_core = []
    for r in range(NCORES):
        rows = slice(KLOC * r, KLOC * (r + 1))
        a_r = ante[rows]                                  # [KLOC, C]
        b_r = bucket[rows]                                # [KLOC, C]

        idx = np.zeros((NPAD,), np.int32)
        idx[:NPAIR] = a_r.reshape(-1).astype(np.int32)
        idx = np.ascontiguousarray(idx.reshape(NT, P).T)  # [P, NT]

        pairsum = (
            TA[rows][:, None, :] + AA[a_r] + FA[b_r] + b_in
        )                                                  # [KLOC, C, H] f32
        psf = np.zeros((NPAD, HP), np.float32)
        psf[:NPAIR, :H] = pairsum.reshape(NPAIR, H)
        # device wants [supertile][unit u][h*SN + q*P + p]; units 150:256 are
        # zero, so ship h=0 fully and only the 22 live rows of h=1
        psf = (
            psf.reshape(NS, ST, P, 2, P)                   # [ns, q, p, h, u]
            .transpose(0, 4, 3, 1, 2)                      # [ns, u, h, q, p]
            .reshape(NS, P, 2 * SN)
        )
        ps = np.ascontiguousarray(psf[:, :, :SN])          # [NS, 128, SN]
        ps1 = np.ascontiguousarray(psf[:, : H - P, SN:])   # [NS, 22, SN]

        feat = np.zeros((NPAD, F), np.float32)
        feat[:NPAIR] = dist_emb[b_r].reshape(NPAIR, F)
        feat = feat.reshape(NT, P, F)

        msum = np.zeros(NPAD, np.float32)
        msum[:NPAIR] = (ms[rows][:, None] + ms[a_r] + b_out[0]).reshape(-1)
        msum = msum.reshape(NS, SN)

        per_core.append(
            dict(
                span=span,
                span_loc=np.ascontiguousarray(span[rows]),
                idx=idx,
                ps=ps,
                ps1=ps1,
                feat=feat,
                msum=msum,
                w3=w3c,
                wh=whc,
                wo=woc,
                bh=bh,
                ident=ident,
            )
        )
    return per_core


def emulate_core(ci):
    """Numpy emulation of the device kernel for one core (layout check)."""
    span, span_loc, idx = ci["span"], ci["span_loc"], ci["idx"]
    ps, feat, msum = ci["ps"], ci["feat"], ci["msum"]
    w3, wh, wo, bh = ci["w3"], ci["wh"], ci["wo"], ci["bh"]
    pair_out = np.zeros((NPAD, W), np.float32)
    sc_out = np.zeros((NS, SN), np.float32)
    # pair tiles
    for t in range(NT):
        g = t * P
        blk = np.zeros((P, WB), np.float32)
        for i in range(g // C, min((g + P - 1) // C, KLOC - 1) + 1):
            a = max(0, i * C - g)
            b = min(P, (i + 1) * C - g)
            if a < b:
                blk[a:b, 0:D] = span_loc[i]
        blk[:, D : 2 * D] = span[idx[:, t]]
        blk[:, 2 * D : 3 * D] = blk[:, 0:D] * blk[:, D : 2 * D]
        pair_out[g : g + P, 0:WB] = blk
        pair_out[g : g + P, WB:] = feat[t]
    # FFNN per supertile
    for ns in range(NS):
        simT = np.zeros((4, P, SN), np.float32)
        for c in range(4):
            for q in range(ST):
                t = ns * ST + q
                simT[c][:, q * P : (q + 1) * P] = pair_out[
                    t * P : (t + 1) * P, 2 * D + CH[c] : 2 * D + CH[c] + P
                ].T
        psfull = np.zeros((2, P, SN), np.float32)
        psfull[0] = ps[ns]
        psfull[1, : H - P] = ci["ps1"][ns]
        h1 = np.zeros((2, P, SN), np.float32)
        for h in range(2):
            acc = np.zeros((P, SN), np.float32)
            for c in range(4):
                acc += w3[c][:, h * P : (h + 1) * P].T @ simT[c]
            acc += psfull[h]
            h1[h] = np.maximum(acc, 0.0)
        h2 = np.zeros((2, P, SN), np.float32)
        for h in range(2):
            acc = np.zeros((P, SN), np.float32)
            for c in range(2):
                acc += wh[c][:, h * P : (h + 1) * P].T @ h1[c]
            h2[h] = np.maximum(acc + bh[:, h : h + 1], 0.0)
        sc = np.zeros((1, SN), np.float32)
        for c in range(2):
            sc += wo[c].T @ h2[c]
        sc_out[ns] = sc[0] + msum[ns]
    return pair_out, sc_out


# ------------------------------------------------------------- bass builder
def build_nc():
    import concourse.bass as bass
    import concourse.bacc as bacc
    import concourse.mybir as mybir
    import concourse.tile as tile

    f32 = mybir.dt.float32
    i32 = mybir.dt.int32
    Relu = mybir.ActivationFunctionType.Relu

    nc = bacc.Bacc(None, target_bir_lowering=False)

    span = nc.dram_tensor("span", [K, D], f32, kind="ExternalInput")
    span_loc = nc.dram_tensor("span_loc", [KLOC, D], f32, kind="ExternalInput")
    idx = nc.dram_tensor("idx", [P, NT], i32, kind="ExternalInput")
    ps = nc.dram_tensor("ps", [NS, P, SN], f32, kind="ExternalInput")
    ps1 = nc.dram_tensor("ps1", [NS, H - P, SN], f32, kind="ExternalInput")
    feat = nc.dram_tensor("feat", [NT, P, F], f32, kind="ExternalInput")
    msum = nc.dram_tensor("msum", [NS, SN], f32, kind="ExternalInput")
    w3 = nc.dram_tensor("w3", [4, P, HP], f32, kind="ExternalInput")
    wh = nc.dram_tensor("wh", [2, P, HP], f32, kind="ExternalInput")
    wo = nc.dram_tensor("wo", [2, P, 1], f32, kind="ExternalInput")
    bh = nc.dram_tensor("bh", [P, 2], f32, kind="ExternalInput")
    ident = nc.dram_tensor("ident", [P, P], f32, kind="ExternalInput")
    pair_out = nc.dram_tensor("pair_out", [NPAD, W], f32, kind="ExternalOutput")
    sc_out = nc.dram_tensor("sc_out", [NS, SN], f32, kind="ExternalOutput")

    with tile.TileContext(nc) as tc:
        with (
            tc.tile_pool(name="const", bufs=1) as cpool,
            tc.tile_pool(name="pair", bufs=4) as ppool,
            tc.tile_pool(name="work", bufs=2) as wpool,
            tc.tile_pool(name="io", bufs=3) as iopool,
            tc.tile_pool(name="pssim", bufs=3, space="PSUM") as ps_sim,
            tc.tile_pool(name="psmm", bufs=1, space="PSUM") as ps_mm,
        ):
            w3_sb = []
            for c in range(4):
                t_ = cpool.tile([P, HP], f32, tag=f"w3_{c}")
                nc.sync.dma_start(t_[:], w3[c])
                w3_sb.append(t_)
            wh_sb = []
            for c in range(2):
                t_ = cpool.tile([P, HP], f32, tag=f"wh_{c}")
                nc.sync.dma_start(t_[:], wh[c])
                wh_sb.append(t_)
            wo_sb = []
            for c in range(2):
                t_ = cpool.tile([P, 1], f32, tag=f"wo_{c}")
                nc.sync.dma_start(t_[:], wo[c])
                wo_sb.append(t_)
            bh_sb = cpool.tile([P, 2], f32, tag="bh")
            nc.sync.dma_start(bh_sb[:], bh[:, :])
            id_sb = cpool.tile([P, P], f32, tag="ident")
            nc.sync.dma_start(id_sb[:], ident[:, :])
            # all antecedent gather indices, loaded once
            idx_sb = cpool.tile([P, NT], i32, tag="idx")
            nc.sync.dma_start(idx_sb[:], idx[:, :])

            for ns in range(NS):
                # ---- one quad tile of 512 pairs ----
                qsb = ppool.tile([P, ST * WB], f32, tag="pair")
                for q in range(ST):
                    t = ns * ST + q
                    g = t * P
                    co = q * WB
                    # zero first if this tile has padded tail rows
                    # (engines need 32-aligned partition bases, so memset
                    # the whole block and let the broadcasts overwrite)
                    if g + P > NPAIR:
                        nc.vector.memset(qsb[:, co : co + D], 0.0)
                    # target rows: broadcast-read DMAs (per span row)
                    for i in range(g // C, min((g + P - 1) // C, KLOC - 1) + 1):
                        a = max(0, i * C - g)
                        b = min(P, (i + 1) * C - g)
                        if a < b:
                            nc.sync.dma_start(
                                qsb[a:b, co : co + D],
                                span_loc[i : i + 1, :].to_broadcast([b - a, D]),
                            )
                    # antecedent rows: indirect gather
                    nc.gpsimd.indirect_dma_start(
                        out=qsb[:, co + D : co + 2 * D],
                        out_offset=None,
                        in_=span[:, :],
                        in_offset=bass.IndirectOffsetOnAxis(
                            ap=idx_sb[:, t : t + 1], axis=0
                        ),
                    )
                # sim = tgt * ante for all four 128-pair blocks in one DVE op
                v = qsb[:].rearrange("p (b x) -> p b x", b=ST)
                nc.vector.tensor_tensor(
                    out=v[:, :, 2 * D : 3 * D],
                    in0=v[:, :, 0:D],
                    in1=v[:, :, D : 2 * D],
                    op=mybir.AluOpType.mult,
                )
                # write all four 128-pair blocks (cols [0, 3D)) in one DMA
                nc.sync.dma_start(
                    pair_out[ns * SN : (ns + 1) * SN, 0:WB].rearrange(
                        "(b p) x -> p b x", p=P
                    ),
                    v[:, :, :],
                )

                # ---- FFNN over the 512-pair supertile ----
                simT_sb = wpool.tile([P, 4 * SN], f32, tag="simT_sb")
                for c in range(4):
                    sps = ps_sim.tile([P, SN], f32, tag="simT")
                    for q in range(ST):
                        nc.tensor.transpose(
                            out=sps[:, q * P : (q + 1) * P],
                            in_=qsb[
                                :, q * WB + 2 * D + CH[c] :
                                q * WB + 2 * D + CH[c] + P
                            ],
                            identity=id_sb[:],
                        )
                    if c % 2 == 0:
                        nc.scalar.copy(
                            out=simT_sb[:, c * SN : (c + 1) * SN], in_=sps[:]
                        )
                    else:
                        nc.vector.tensor_copy(
                            out=simT_sb[:, c * SN : (c + 1) * SN], in_=sps[:]
                        )

                ps_sb = wpool.tile([P, 2 * SN], f32, tag="ps")
                nc.vector.memset(ps_sb[:, SN:], 0.0)
                nc.sync.dma_start(ps_sb[:, 0:SN], ps[ns])
                nc.sync.dma_start(ps_sb[0 : H - P, SN:], ps1[ns])

                h1_ps = ps_mm.tile([P, 2 * SN], f32, tag="h1")
                for h in range(2):
                    o = h1_ps[:, h * SN : (h + 1) * SN]
                    for c in range(4):
                        nc.tensor.matmul(
                            o,
                            lhsT=w3_sb[c][:, h * P : (h + 1) * P],
                            rhs=simT_sb[:, c * SN : (c + 1) * SN],
                            start=(c == 0),
                            stop=(c == 3),
                        )
                h1a_sb = wpool.tile([P, 2 * SN], f32, tag="h1a")
                nc.vector.tensor_add(out=h1a_sb[:], in0=h1_ps[:], in1=ps_sb[:])
                h1_sb = wpool.tile([P, 2 * SN], f32, tag="h1s")
                nc.vector.tensor_scalar_max(h1_sb[:], h1a_sb[:], 0.0)

                h2_ps = ps_mm.tile([P, 2 * SN], f32, tag="h2")
                for h in range(2):
                    o = h2_ps[:, h * SN : (h + 1) * SN]
                    for c in range(2):
                        nc.tensor.matmul(
                            o,
                            lhsT=wh_sb[c][:, h * P : (h + 1) * P],
                            rhs=h1_sb[:, c * SN : (c + 1) * SN],
                            start=(c == 0),
                            stop=(c == 1),
                        )
                h2_sb = wpool.tile([P, 2 * SN], f32, tag="h2s")
                for h in range(2):
                    nc.scalar.activation(
                        out=h2_sb[:, h * SN : (h + 1) * SN],
                        in_=h2_ps[:, h * SN : (h + 1) * SN],
                        func=Relu,
                        bias=bh_sb[:, h : h + 1],
                    )

                ms_sb = iopool.tile([1, SN], f32, tag="ms")
                nc.sync.dma_start(ms_sb[:], msum[ns : ns + 1, :])
                sc_ps = ps_mm.tile([1, SN], f32, tag="sc")
                for c in range(2):
                    nc.tensor.matmul(
                        sc_ps[:],
                        lhsT=wo_sb[c][:],
                        rhs=h2_sb[:, c * SN : (c + 1) * SN],
                        start=(c == 0),
                        stop=(c == 1),
                    )
                sc_sb = iopool.tile([1, SN], f32, tag="sc_sb")
                nc.vector.tensor_add(out=sc_sb[:], in0=sc_ps[:], in1=ms_sb[:])
                nc.sync.dma_start(sc_out[ns : ns + 1, :], sc_sb[:])

            # distance features: single DRAM->DRAM strided copy for all tiles
            # (many tiny descriptors -- keep it at the end so it fills DMA
            # idle slack instead of blocking the pipeline start)
            nc.sync.dma_start(
                pair_out[:, WB:],
                feat[:, :, :].rearrange("t p f -> (t p) f"),
            )

    nc.finalize()
    return nc


# ------------------------------------------------------------------ driver
def kernel(**inputs):
    from concourse.bass_utils import run_bass_kernel_spmd

    per_core = host_prep(inputs)

    if "nc" not in _NC_CACHE:
        _NC_CACHE["nc"] = build_nc()
    nc = _NC_CACHE["nc"]

    res = run_bass_kernel_spmd(nc, per_core, core_ids=list(range(NCORES)))
    results = res.results
    kernel.last_run = res  # for test harness introspection (exec_time_ns)

    scores = np.empty((K, C), np.float32)
    pair_emb = np.empty((K, C, W), np.float32)
    for r in range(NCORES):
        rows = slice(KLOC * r, KLOC * (r + 1))
        pair_emb[rows] = results[r]["pair_out"][:NPAIR].reshape(KLOC, C, W)
        scores[rows] = (
            results[r]["sc_out"].reshape(NPAD)[:NPAIR].reshape(KLOC, C)
        )
    antecedent_emb = pair_emb[:, :, D : 2 * D]
    return scores, antecedent_emb, pair_emb
